# revision 29
# baseline (speedup 1.0000x reference)
"""F1-score (histogram_binning) Trainium2 Bass kernel.

Computes: pred = argmax(y_pred, axis=1); cm = confusion_matrix(y_true, pred);
then the scalar F1 epilogue of the reference.

Strategy (fp16 + sorted-by-class data parallel over 8 cores):
  - Host: cast y_pred to fp16 (verified offline: shifts F1 by only 5.6e-4
    relative -- tolerance is 2e-2) and stable-sort samples by true class so
    that PARTITION index == true class on every core (class c's samples are
    split across the 8 cores; each core holds up to F=1056 of them, padded
    with a known row [1,0,...,0] whose argmax is 0).  With that layout the
    confusion-matrix row index IS the partition index, so the matmul lhsT
    is a constant identity -- no per-sample one-hot of the labels is ever
    built or streamed.  fp16 halves the HBM traffic: 33MB/core (~95us).
  - Per block [128 part x G=32 samples x 128 classes] (fp16 on DVE = 2x):
      DVE:  max tree m64/m32/m16/m8 (tensor_tensor max) + reduce_max
      DVE:  rmax2 = rmax duplicated in adjacent pairs (enables the
            pair-packed 2x_1P broadcast read below)
      DVE:  oh = is_ge(x, rmax2-pairs) for slots 0..J-1, one packed TT
      ACT:  s  = Sign(rmax - x) in {0,1} for slots J..31 (bias=rmax)
            (last block: DVE computes those slots as is_lt instead, so the
            trailing engine at the end of the pipeline is DVE, not ACT)
      PE:   8 wide matmuls (identity lhsT, rhs = 4 adjacent slots
            = [128,512] fp16) accumulating into all 8 PSUM banks.
  - Epilogue: DVE+ACT copy the 8 PSUM banks to SBUF as fp16 (counts are
    integers <= 2048, exact), 8 small DMAs out.
    Host: cm = sum_k OH_k + 8*(32-J)*33 - sum_k S_k ; cm[:,0] -= (8F - n_c);
    then the scalar F1 epilogue.

Measured 158.2us/core HW exec (vs 272.9us baseline): DVE ~136us busy
(bound), ACT ~135us, DMA ~95us, PE ~83us, plus ~10us DMA-queue bringup
and ~7us drain/postamble.  Rel err vs the fp32 reference: 5.6e-4.
"""

import sys

import numpy as np

sys.path.insert(0, "/opt/trn_rl_repo")

import concourse.bacc as bacc  # noqa: E402
import concourse.bass as bass  # noqa: E402
import concourse.tile as tile  # noqa: E402
from concourse import mybir  # noqa: E402
from concourse.bass_utils import run_bass_kernel_spmd  # noqa: E402

N_CORES = 8
N_SAMPLES = 1048576
C = 128
EPS = 1e-07
P = 128  # partitions == true-class index
F = 1056  # sample slots per partition per core (8*F >= max class count)
G = 32  # samples per block
N_BLOCKS = F // G  # 33
J = 19  # slots handled by DVE is_ge (oh-kind); slots J..31 are s-kind (ACT)


def build_program():
    nc = bacc.Bacc("TRN2")

    f16 = mybir.dt.float16
    x_t = nc.dram_tensor("x", [P, F, C], f16, kind="ExternalInput")
    ident_t = nc.dram_tensor("ident", [P, C], f16, kind="ExternalInput")
    out_t = nc.dram_tensor("out", [C, G * C], f16, kind="ExternalOutput")

    xs = x_t[:].rearrange("p (b g) c -> p b g c", b=N_BLOCKS, g=G)

    with tile.TileContext(nc) as tc:
        with (
            tc.tile_pool(name="consts", bufs=1) as consts,
            tc.tile_pool(name="xp", bufs=8) as xp,
            tc.tile_pool(name="mp", bufs=4) as mp,
            tc.tile_pool(name="ohp", bufs=6) as ohp,
            tc.tile_pool(name="small", bufs=8) as small,
            tc.tile_pool(name="psum", bufs=1, space="PSUM") as psum_pool,
            tc.tile_pool(name="outp", bufs=1) as outp,
        ):
            ident_sb = consts.tile([P, C], f16)
            nc.gpsimd.dma_start(out=ident_sb, in_=ident_t[:])

            acc = [
                psum_pool.tile([C, 4 * C], mybir.dt.float32, tag=f"acc{q}", name=f"acc{q}")
                for q in range(G // 4)
            ]

            for b in range(N_BLOCKS):
                x = xp.tile([P, G, C], f16)
                m64 = mp.tile([P, G, 64], f16)
                if b == 0:
                    # quarter the first block's DMA + fold1 so DVE starts as
                    # soon as the first 256KB lands (shaves pipeline fill)
                    for mb in range(4):
                        sl = slice(8 * mb, 8 * (mb + 1))
                        nc.sync.dma_start(out=x[:, sl, :], in_=xs[:, 0, sl])
                    for mb in range(4):
                        sl = slice(8 * mb, 8 * (mb + 1))
                        nc.vector.tensor_tensor(
                            out=m64[:, sl, :],
                            in0=x[:, sl, 0:64], in1=x[:, sl, 64:128],
                            op=mybir.AluOpType.max,
                        )
                else:
                    nc.sync.dma_start(out=x, in_=xs[:, b])
                    # fp16 max tree on DVE: all tensor_tensor at 2x_1P
                    nc.vector.tensor_tensor(
                        out=m64, in0=x[:, :, 0:64], in1=x[:, :, 64:128],
                        op=mybir.AluOpType.max,
                    )
                m32 = mp.tile([P, G, 32], f16, tag="m32")
                nc.vector.tensor_tensor(
                    out=m32, in0=m64[:, :, 0:32], in1=m64[:, :, 32:64],
                    op=mybir.AluOpType.max,
                )
                m16 = mp.tile([P, G, 16], f16, tag="m16")
                nc.vector.tensor_tensor(
                    out=m16, in0=m32[:, :, 0:16], in1=m32[:, :, 16:32],
                    op=mybir.AluOpType.max,
                )
                m8 = mp.tile([P, G, 8], f16, tag="m8")
                nc.vector.tensor_tensor(
                    out=m8, in0=m16[:, :, 0:8], in1=m16[:, :, 8:16],
                    op=mybir.AluOpType.max,
                )
                rmax = small.tile([P, G], f16)
                nc.vector.tensor_reduce(
                    out=rmax, in_=m8,
                    axis=mybir.AxisListType.X, op=mybir.AluOpType.max,
                )
                # duplicate each max into an adjacent pair: rmax2[p, 2g] =
                # rmax2[p, 2g+1] = rmax[p, g] (for packed-pair broadcast);
                # two strided-write SBUF->SBUF DMAs keep this off the DVE
                rmax2 = small.tile([P, 2 * G], f16, tag="rmax2")
                for half in range(2):
                    nc.sync.dma_start(
                        out=bass.AP(
                            tensor=rmax2.tensor, offset=rmax2.offset + half,
                            ap=[[2 * G, P], [2, G]],
                        ),
                        in_=rmax[:, :],
                    )

                ohs = ohp.tile([P, G, C], f16)
                # slots 0..J-1: oh = (x >= rowmax), one pair-packed 2x TT
                nc.vector.tensor_tensor(
                    out=bass.AP(
                        tensor=ohs.tensor, offset=ohs.offset,
                        ap=[[G * C, P], [C, J], [2, 64], [1, 2]],
                    ),
                    in0=bass.AP(
                        tensor=x.tensor, offset=x.offset,
                        ap=[[G * C, P], [C, J], [2, 64], [1, 2]],
                    ),
                    in1=bass.AP(
                        tensor=rmax2.tensor, offset=rmax2.offset,
                        ap=[[2 * G, P], [2, J], [0, 64], [1, 2]],
                    ),
                    op=mybir.AluOpType.is_ge,
                )
                if b == N_BLOCKS - 1:
                    # final block: keep the critical path on DVE -- compute
                    # the s-kind slots as is_lt (same {0,1} mask as Sign)
                    nc.vector.tensor_tensor(
                        out=bass.AP(
                            tensor=ohs.tensor, offset=ohs.offset + J * C,
                            ap=[[G * C, P], [C, G - J], [2, 64], [1, 2]],
                        ),
                        in0=bass.AP(
                            tensor=x.tensor, offset=x.offset + J * C,
                            ap=[[G * C, P], [C, G - J], [2, 64], [1, 2]],
                        ),
                        in1=bass.AP(
                            tensor=rmax2.tensor, offset=rmax2.offset + 2 * J,
                            ap=[[2 * G, P], [2, G - J], [0, 64], [1, 2]],
                        ),
                        op=mybir.AluOpType.is_lt,
                    )
                else:
                    # slots J..31: s = Sign(rowmax - x) in {0,1} on ACT
                    for g in range(J, G):
                        nc.scalar.activation(
                            out=ohs[:, g, :],
                            in_=x[:, g, :],
                            func=mybir.ActivationFunctionType.Sign,
                            bias=rmax[:, g : g + 1],
                            scale=-1.0,
                        )

                first = b == 0
                last = b == N_BLOCKS - 1
                for q in range(G // 4):
                    nc.tensor.matmul(
                        acc[q],
                        lhsT=ident_sb,
                        rhs=ohs[:, 4 * q : 4 * q + 4, :],
                        start=first,
                        stop=last,
                    )

            res_sb = outp.tile([C, G * C], f16)
            for q in range(G // 4):
                sl = res_sb[:, 4 * C * q : 4 * C * (q + 1)]
                if q % 2 == 0:
                    nc.vector.tensor_copy(out=sl, in_=acc[q])
                else:
                    nc.scalar.copy(out=sl, in_=acc[q])
                nc.sync.dma_start(
                    out=out_t[:, 4 * C * q : 4 * C * (q + 1)], in_=sl
                )

    nc.finalize()
    return nc


_PROGRAM = None


def _get_program():
    global _PROGRAM
    if _PROGRAM is None:
        _PROGRAM = build_program()
    return _PROGRAM


def _shard_inputs(y_pred, y_true):
    """Cast to fp16 and sort by true class; partition p holds class-p rows."""
    y_pred = np.asarray(y_pred)
    y_true = np.asarray(y_true).astype(np.int64)
    n = y_true.shape[0]

    cnt = np.bincount(y_true, minlength=C)
    assert cnt.max() <= N_CORES * F, f"class count {cnt.max()} exceeds capacity"
    order = np.argsort(y_true, kind="stable")
    starts = np.zeros(C, dtype=np.int64)
    starts[1:] = np.cumsum(cnt)[:-1]

    # idx[k, c, f] = sample row (or n for the pad row)
    idx = np.full((N_CORES, C, F), n, dtype=np.int64)
    for c in range(C):
        m, s0 = int(cnt[c]), int(starts[c])
        q, r = divmod(m, N_CORES)
        off = 0
        for k in range(N_CORES):
            take = q + (1 if k < r else 0)
            idx[k, c, :take] = order[s0 + off : s0 + off + take]
            off += take

    y16 = y_pred.astype(np.float16)
    pad_row = np.zeros((1, C), dtype=np.float16)
    pad_row[0, 0] = 1.0  # argmax = 0, decisively
    y_ext = np.concatenate([y16, pad_row], axis=0)

    ident = np.eye(C, dtype=np.float16)
    in_maps = []
    for k in range(N_CORES):
        xk = y_ext[idx[k].reshape(-1)].reshape(P, F, C)
        in_maps.append({"x": xk, "ident": ident})
    return in_maps, cnt


def _epilogue(cm):
    cm = cm.astype(np.float32)
    TP = np.diagonal(cm)
    FP = (C - 1) * cm[:, 1] + cm[:, 0]
    FN = (C - 1) * cm[1, :] + cm[0, :]
    eps = np.float32(EPS)
    sensitivity = np.mean(TP / (TP + FN + eps), dtype=np.float32)
    precision = np.mean(TP / (TP + FP + eps), dtype=np.float32)
    f1 = np.float32(2.0) * (precision * sensitivity / (precision + sensitivity + eps))
    return np.asarray(f1, dtype=np.float32)


def run_on_device(y_pred, y_true, **kwargs):
    """Run the bass kernel on 8 cores; returns (cm_total, results_obj)."""
    nc = _get_program()
    in_maps, cnt = _shard_inputs(y_pred, y_true)
    res = run_bass_kernel_spmd(nc, in_maps, core_ids=list(range(N_CORES)), **kwargs)

    n_s_slots = (G - J) * N_BLOCKS  # s-kind slots per partition per core
    cm = np.zeros((C, C), dtype=np.float64)
    cm += N_CORES * n_s_slots  # the "+1" part of (1 - s) for every s-slot sample
    for r in res.results:
        out = r["out"].astype(np.float64)  # [C, G*C]
        chunks = out.reshape(C, G, C)
        oh = chunks[:, 0:J, :].sum(axis=1)  # slots 0..J-1 (is_ge one-hots)
        s = chunks[:, J:G, :].sum(axis=1)  # slots J..31  (s masks)
        cm += oh - s
    # every pad slot (both kinds) contributed exactly e_0 to cm's row
    cm[:, 0] -= N_CORES * F - cnt
    return cm, res


def kernel(y_pred, y_true):
    cm, _ = run_on_device(y_pred, y_true)
    return _epilogue(cm)


# revision 30
# speedup vs baseline: 2.1170x; 2.1170x over previous
"""F1-score (histogram_binning) Trainium2 Bass kernel.

Computes: pred = argmax(y_pred, axis=1); cm = confusion_matrix(y_true, pred);
then the scalar F1 epilogue of the reference.

Strategy (fp16 + sorted-by-class data parallel over 8 cores):
  - Host: cast y_pred to fp16 (verified offline: shifts F1 by only 5.6e-4
    relative -- tolerance is 2e-2) and stable-sort samples by true class so
    that PARTITION index == true class on every core (class c's samples are
    split across the 8 cores; each core holds up to F=1056 of them, padded
    with a known row [1,0,...,0] whose argmax is 0).  With that layout the
    confusion-matrix row index IS the partition index, so the matmul lhsT
    is a constant identity -- no per-sample one-hot of the labels is ever
    built or streamed.  fp16 halves the HBM traffic: 33MB/core (~95us).
  - Per block [128 part x G=32 samples x 128 classes] (fp16 on DVE = 2x):
      DVE:  max tree m64/m32/m16/m8 (tensor_tensor max) + reduce_max
      DVE:  rmax2 = rmax duplicated in adjacent pairs (enables the
            pair-packed 2x_1P broadcast read below)
      DVE:  oh = is_ge(x, rmax2-pairs) for slots 0..J-1, one packed TT
      ACT:  s  = Sign(rmax - x) in {0,1} for slots J..31 (bias=rmax)
            (last block: DVE computes those slots as is_lt instead, so the
            trailing engine at the end of the pipeline is DVE, not ACT)
      PE:   8 wide matmuls (identity lhsT, rhs = 4 adjacent slots
            = [128,512] fp16) accumulating into all 8 PSUM banks.
  - Epilogue: DVE+ACT copy the 8 PSUM banks to SBUF as fp16 (counts are
    integers <= 2048, exact), 8 small DMAs out.
    Host: cm = sum_k OH_k + 8*(32-J)*33 - sum_k S_k ; cm[:,0] -= (8F - n_c);
    then the scalar F1 epilogue.

Measured 158.2us/core HW exec (vs 272.9us baseline): DVE ~136us busy
(bound), ACT ~135us, DMA ~95us, PE ~83us, plus ~10us DMA-queue bringup
and ~7us drain/postamble.  Rel err vs the fp32 reference: 5.6e-4.
"""

import sys

import numpy as np

sys.path.insert(0, "/opt/trn_rl_repo")

import concourse.bacc as bacc  # noqa: E402
import concourse.bass as bass  # noqa: E402
import concourse.tile as tile  # noqa: E402
from concourse import mybir  # noqa: E402
from concourse.bass_utils import run_bass_kernel_spmd  # noqa: E402

N_CORES = 8
N_SAMPLES = 1048576
C = 128
EPS = 1e-07
P = 128  # partitions == true-class index
F = 1056  # sample slots per partition per core (8*F >= max class count)
G = 32  # samples per block
N_BLOCKS = F // G  # 33
J = 18  # slots handled by DVE is_ge (oh-kind); slots J..31 are s-kind (ACT)


def build_program():
    nc = bacc.Bacc("TRN2")

    f16 = mybir.dt.float16
    x_t = nc.dram_tensor("x", [P, F, C], f16, kind="ExternalInput")
    ident_t = nc.dram_tensor("ident", [P, C], f16, kind="ExternalInput")
    out_t = nc.dram_tensor("out", [C, G * C], f16, kind="ExternalOutput")

    xs = x_t[:].rearrange("p (b g) c -> p b g c", b=N_BLOCKS, g=G)

    with tile.TileContext(nc) as tc:
        with (
            tc.tile_pool(name="consts", bufs=1) as consts,
            tc.tile_pool(name="xp", bufs=8) as xp,
            tc.tile_pool(name="mp", bufs=4) as mp,
            tc.tile_pool(name="ohp", bufs=6) as ohp,
            tc.tile_pool(name="small", bufs=8) as small,
            tc.tile_pool(name="psum", bufs=1, space="PSUM") as psum_pool,
            tc.tile_pool(name="outp", bufs=1) as outp,
        ):
            ident_sb = consts.tile([P, C], f16)
            nc.gpsimd.dma_start(out=ident_sb, in_=ident_t[:])

            acc = [
                psum_pool.tile([C, 4 * C], mybir.dt.float32, tag=f"acc{q}", name=f"acc{q}")
                for q in range(G // 4)
            ]

            for b in range(N_BLOCKS):
                x = xp.tile([P, G, C], f16)
                m64 = mp.tile([P, G, 64], f16)
                if b == 0:
                    # quarter the first block's DMA + fold1 so DVE starts as
                    # soon as the first 256KB lands (shaves pipeline fill)
                    for mb in range(4):
                        sl = slice(8 * mb, 8 * (mb + 1))
                        nc.sync.dma_start(out=x[:, sl, :], in_=xs[:, 0, sl])
                    for mb in range(4):
                        sl = slice(8 * mb, 8 * (mb + 1))
                        nc.vector.tensor_tensor(
                            out=m64[:, sl, :],
                            in0=x[:, sl, 0:64], in1=x[:, sl, 64:128],
                            op=mybir.AluOpType.max,
                        )
                else:
                    nc.sync.dma_start(out=x, in_=xs[:, b])
                    # fp16 max tree on DVE: all tensor_tensor at 2x_1P
                    nc.vector.tensor_tensor(
                        out=m64, in0=x[:, :, 0:64], in1=x[:, :, 64:128],
                        op=mybir.AluOpType.max,
                    )
                m32 = mp.tile([P, G, 32], f16, tag="m32")
                nc.vector.tensor_tensor(
                    out=m32, in0=m64[:, :, 0:32], in1=m64[:, :, 32:64],
                    op=mybir.AluOpType.max,
                )
                m16 = mp.tile([P, G, 16], f16, tag="m16")
                nc.vector.tensor_tensor(
                    out=m16, in0=m32[:, :, 0:16], in1=m32[:, :, 16:32],
                    op=mybir.AluOpType.max,
                )
                m8 = mp.tile([P, G, 8], f16, tag="m8")
                nc.vector.tensor_tensor(
                    out=m8, in0=m16[:, :, 0:8], in1=m16[:, :, 8:16],
                    op=mybir.AluOpType.max,
                )
                rmax = small.tile([P, G], mybir.dt.float32)
                nc.vector.tensor_reduce(
                    out=rmax, in_=m8,
                    axis=mybir.AxisListType.X, op=mybir.AluOpType.max,
                )
                # duplicate each max into an adjacent pair: rmax2[p, 2g] =
                # rmax2[p, 2g+1] = rmax[p, g] (for packed-pair broadcast)
                rmax2 = small.tile([P, 2 * G], f16, tag="rmax2")
                nc.vector.tensor_copy(
                    out=bass.AP(
                        tensor=rmax2.tensor, offset=rmax2.offset,
                        ap=[[2 * G, P], [2, G], [1, 2]],
                    ),
                    in_=bass.AP(
                        tensor=rmax.tensor, offset=rmax.offset,
                        ap=[[G, P], [1, G], [0, 2]],
                    ),
                )

                ohs = ohp.tile([P, G, C], f16)
                # slots 0..J-1: oh = (x >= rowmax), one pair-packed 2x TT
                nc.vector.tensor_tensor(
                    out=bass.AP(
                        tensor=ohs.tensor, offset=ohs.offset,
                        ap=[[G * C, P], [C, J], [2, 64], [1, 2]],
                    ),
                    in0=bass.AP(
                        tensor=x.tensor, offset=x.offset,
                        ap=[[G * C, P], [C, J], [2, 64], [1, 2]],
                    ),
                    in1=bass.AP(
                        tensor=rmax2.tensor, offset=rmax2.offset,
                        ap=[[2 * G, P], [2, J], [0, 64], [1, 2]],
                    ),
                    op=mybir.AluOpType.is_ge,
                )
                if b == N_BLOCKS - 1:
                    # final block: keep the critical path on DVE -- compute
                    # the s-kind slots as is_lt (same {0,1} mask as Sign)
                    nc.vector.tensor_tensor(
                        out=bass.AP(
                            tensor=ohs.tensor, offset=ohs.offset + J * C,
                            ap=[[G * C, P], [C, G - J], [2, 64], [1, 2]],
                        ),
                        in0=bass.AP(
                            tensor=x.tensor, offset=x.offset + J * C,
                            ap=[[G * C, P], [C, G - J], [2, 64], [1, 2]],
                        ),
                        in1=bass.AP(
                            tensor=rmax2.tensor, offset=rmax2.offset + 2 * J,
                            ap=[[2 * G, P], [2, G - J], [0, 64], [1, 2]],
                        ),
                        op=mybir.AluOpType.is_lt,
                    )
                else:
                    # slots J..31: s = Sign(rowmax - x) in {0,1} on ACT
                    for g in range(J, G):
                        nc.scalar.activation(
                            out=ohs[:, g, :],
                            in_=x[:, g, :],
                            func=mybir.ActivationFunctionType.Sign,
                            bias=rmax[:, g : g + 1],
                            scale=-1.0,
                        )

                first = b == 0
                last = b == N_BLOCKS - 1
                for q in range(G // 4):
                    nc.tensor.matmul(
                        acc[q],
                        lhsT=ident_sb,
                        rhs=ohs[:, 4 * q : 4 * q + 4, :],
                        start=first,
                        stop=last,
                    )

            res_sb = outp.tile([C, G * C], f16)
            for q in range(G // 4):
                sl = res_sb[:, 4 * C * q : 4 * C * (q + 1)]
                if q % 2 == 0:
                    nc.vector.tensor_copy(out=sl, in_=acc[q])
                else:
                    nc.scalar.copy(out=sl, in_=acc[q])
                nc.sync.dma_start(
                    out=out_t[:, 4 * C * q : 4 * C * (q + 1)], in_=sl
                )

    nc.finalize()
    return nc


_PROGRAM = None


def _get_program():
    global _PROGRAM
    if _PROGRAM is None:
        _PROGRAM = build_program()
    return _PROGRAM


def _shard_inputs(y_pred, y_true):
    """Cast to fp16 and sort by true class; partition p holds class-p rows."""
    y_pred = np.asarray(y_pred)
    y_true = np.asarray(y_true).astype(np.int64)
    n = y_true.shape[0]

    cnt = np.bincount(y_true, minlength=C)
    assert cnt.max() <= N_CORES * F, f"class count {cnt.max()} exceeds capacity"
    order = np.argsort(y_true, kind="stable")
    starts = np.zeros(C, dtype=np.int64)
    starts[1:] = np.cumsum(cnt)[:-1]

    # idx[k, c, f] = sample row (or n for the pad row)
    idx = np.full((N_CORES, C, F), n, dtype=np.int64)
    for c in range(C):
        m, s0 = int(cnt[c]), int(starts[c])
        q, r = divmod(m, N_CORES)
        off = 0
        for k in range(N_CORES):
            take = q + (1 if k < r else 0)
            idx[k, c, :take] = order[s0 + off : s0 + off + take]
            off += take

    y16 = y_pred.astype(np.float16)
    pad_row = np.zeros((1, C), dtype=np.float16)
    pad_row[0, 0] = 1.0  # argmax = 0, decisively
    y_ext = np.concatenate([y16, pad_row], axis=0)

    ident = np.eye(C, dtype=np.float16)
    in_maps = []
    for k in range(N_CORES):
        xk = y_ext[idx[k].reshape(-1)].reshape(P, F, C)
        in_maps.append({"x": xk, "ident": ident})
    return in_maps, cnt


def _epilogue(cm):
    cm = cm.astype(np.float32)
    TP = np.diagonal(cm)
    FP = (C - 1) * cm[:, 1] + cm[:, 0]
    FN = (C - 1) * cm[1, :] + cm[0, :]
    eps = np.float32(EPS)
    sensitivity = np.mean(TP / (TP + FN + eps), dtype=np.float32)
    precision = np.mean(TP / (TP + FP + eps), dtype=np.float32)
    f1 = np.float32(2.0) * (precision * sensitivity / (precision + sensitivity + eps))
    return np.asarray(f1, dtype=np.float32)


def run_on_device(y_pred, y_true, **kwargs):
    """Run the bass kernel on 8 cores; returns (cm_total, results_obj)."""
    nc = _get_program()
    in_maps, cnt = _shard_inputs(y_pred, y_true)
    res = run_bass_kernel_spmd(nc, in_maps, core_ids=list(range(N_CORES)), **kwargs)

    n_s_slots = (G - J) * N_BLOCKS  # s-kind slots per partition per core
    cm = np.zeros((C, C), dtype=np.float64)
    cm += N_CORES * n_s_slots  # the "+1" part of (1 - s) for every s-slot sample
    for r in res.results:
        out = r["out"].astype(np.float64)  # [C, G*C]
        chunks = out.reshape(C, G, C)
        oh = chunks[:, 0:J, :].sum(axis=1)  # slots 0..J-1 (is_ge one-hots)
        s = chunks[:, J:G, :].sum(axis=1)  # slots J..31  (s masks)
        cm += oh - s
    # every pad slot (both kinds) contributed exactly e_0 to cm's row
    cm[:, 0] -= N_CORES * F - cnt
    return cm, res


def kernel(y_pred, y_true):
    cm, _ = run_on_device(y_pred, y_true)
    return _epilogue(cm)
